# revision 41
# baseline (speedup 1.0000x reference)
"""Trainium2 Bass kernel for the 3-expert MoE routing MLP.

Reference computation (B=1M rows):
    y1  = tanh(x @ w1 - b1)                     # [B, 8]
    h_k = sigmoid(y1 @ wa_k - ba_k)             # [B, 16] for experts k=0,1,2
    e_k = h_k @ wb_k - bb_k                     # [B, 32]
    y   = e_{u[b]}  per row b

Strategy (pure data parallel over 8 cores, ~125000 rows/core):

  * The HOST routes: each core's rows are stably partitioned by expert id
    into 3 segments padded to N_G rows (N_G = 42000 for the seed-0 input,
    0.8% pad).  The device then runs only the SELECTED expert per row as
    dense matmuls -- no masking, no onehot, no u upload -- and the host
    inverts the permutation on unpack.

  * x is shipped as float8_e4m3 scaled by XS=2 (w1 by WS=8, both folded
    out via the ACT scale), which halves input DMA vs fp16, and the trunk
    matmul runs in fp8 DoubleRow mode: two K=128 planes accumulate in one
    pass (effective K=256), so each PSUM column holds FOUR rows and the
    trunk costs 0.25 PE cols/row.  Final rel err ~1e-2 (sim-verified).

  * Per 8*cs-row block (cs=500 free cols; 250 for the segment-tail block),
    with blocks paired into "supersteps" that share DMA tiles and trunk
    PSUM (DoubleRow dst must sit at PSUM partition 0 -- ISA restriction --
    so the pair's trunk results stack in the free dim at cols 0/512):
      - 2 DoubleRow trunk matmuls per block (M=64, zero-padded cols) into
        S_a/S_b[64, 1024]: T_y partition 64h+8s'+f = y1 feature f of slot
        s = 4h+s'; slot s of column t is row 4cs*h + 4t + s'.
      - 2 ACT tanh(S/16 - b1) per superstep -> T_y[0:64] / T_y[64:128].
      - mmH: lhsT [128,128] block-diag wa_k (8 slots x 16 hidden), ONE ACT
        sigmoid(+ -ba_k) -> T_g[128, cs] fp16 (slot s at partitions 16s+).
      - 2 mmF: lhsT [64,128] block-diag wb_k (4 slots x 32 out) over
        T_g[0:64] and T_g[64:128] (weights duplicated at partitions 64-127
        so tile_position rows match), -> 2 PSUM tiles [128, cs].
      - 2 DVE tensor_scalar subtract bb_k: PSUM f32 -> out fp16 (the last
        two supersteps route half of these to the then-idle ACT engine).
    PE: ~5*cs cycles per 8*cs rows; ACT 3 ops, DVE 2 ops per 2 blocks.

  * DMA per core: 8.1 MB in (e4m3) + 8.1 MB out (fp16) -- the roofline.
"""

import math

import numpy as np
import ml_dtypes

import concourse.bass as bass
import concourse.tile as tile
from concourse import mybir
from concourse.bass_utils import run_bass_kernel_spmd

F32 = mybir.dt.float32
F16 = mybir.dt.float16
F8 = mybir.dt.float8e4
E4 = ml_dtypes.float8_e4m3

N_CORES = 8
B = 1_000_000
IN = 64
OUT = 32
B_C = B // N_CORES          # rows per core
N_G_MIN = 42000             # default per-expert segment size (pad target)
XS = 2.0                    # x pre-scale for e4m3 quantization
WS = 8.0                    # w1 pre-scale for e4m3 quantization

# module knobs for the test harness (kernel() itself never reads files)
_TRACE = False
_LAST_RES = None


def _blocks(n_g):
    """Per-segment block list: [(expert, cs)] with 8*cs rows per block."""
    assert n_g % 2000 == 0
    out = []
    for k in range(3):
        rem = n_g
        while rem >= 4000:
            out.append((k, 500))
            rem -= 4000
        if rem:
            assert rem == 2000
            out.append((k, 250))
    return out


def _binfo(n_g):
    """[(expert, cs, x_col0, y_col0)] for every block, in emission order.

    x2 and yT share the column index m = row//4 (x2 holds 2 fp8 planes)."""
    info = []
    m0 = 0
    for k, cs in _blocks(n_g):
        info.append((k, cs, m0, m0))
        m0 += 2 * cs
    return info


def _tinfo(n_g):
    """DMA tile plan: pairs of big blocks share one x tile / out tile.

    Returns (tiles, owner) where tiles[t] = (m0, mlen) and owner[s] =
    (t, off, is_last_block_of_tile) for every block s of _binfo."""
    info = _binfo(n_g)
    tiles = []
    owner = []
    s = 0
    while s < len(info):
        k, cs, m0, y0 = info[s]
        if (cs == 500 and s + 1 < len(info) and info[s + 1][1] == 500
                and info[s + 1][0] == k):
            tiles.append((m0, 4 * cs))
            owner.append((len(tiles) - 1, 0, False))
            owner.append((len(tiles) - 1, 2 * cs, True))
            s += 2
        else:
            tiles.append((m0, 2 * cs))
            owner.append((len(tiles) - 1, 0, True))
            s += 1
    return tiles, owner


def _pack_weights(w1, b1, w2, b2, w3, b3, w4, b4, w5, b5, w6, b6, w7, b7):
    f32 = np.float32
    wa_list = [w2, w4, w6]
    ba_list = [b2, b4, b6]
    wb_list = [w3, w5, w7]
    bb_list = [b3, b5, b7]

    # trunk DoubleRow lhsT [128, 2, 64] e4m3 (cols 32-63 zero; the two
    # M=64 matmuls must both target PSUM partition 0 -- DoubleRow ISA
    # restriction -- so they write col-ranges of shared [64, 1024] tiles):
    #   W[64a+f, i, 8s+g] = WS*w1[f, g] where s = 2i+a, else 0
    wa8 = np.zeros((128, 2, 64), f32)
    for i in range(2):
        for a in range(2):
            s = 2 * i + a
            wa8[64 * a:64 * a + 64, i, 8 * s:8 * s + 8] = WS * w1
    wa8 = wa8.astype(E4)

    # mmH lhsT [128, 128] per expert: T_y row 64h+8s'+f (slot s = 4h+s')
    # -> col 16s+j = wa_k[f, j]; rows 32-63 / 96-127 are zero.
    wh16 = np.zeros((128, 3 * 128), np.float16)
    for k in range(3):
        for s in range(8):
            r0 = 64 * (s // 4) + 8 * (s % 4)
            wh16[r0:r0 + 8, 128 * k + 16 * s:128 * k + 16 * s + 16] = \
                wa_list[k].astype(np.float16)

    # mmF lhsT [64, 128] per expert: row 16a+j -> col block of slot a;
    # duplicated at partitions 64-127 for the second (upper-half) matmul.
    wf16 = np.zeros((128, 3 * 128), np.float16)
    for k in range(3):
        for a in range(4):
            blk = wb_list[k].astype(np.float16)
            wf16[16 * a:16 * a + 16, 128 * k + 32 * a:128 * k + 32 * a + 32] = blk
            wf16[64 + 16 * a:64 + 16 * a + 16,
                 128 * k + 32 * a:128 * k + 32 * a + 32] = blk

    # biases [128, 10] f32: col0 trunk -b1 (rows 64h+8s'+f); col 1+k mmH
    # -ba_k (rows 16s+j); col 4+k mmF bb_k (rows 32a+o); col 7+k -bb_k
    # (for ACT Identity adds in the drain tail)
    bp = np.zeros((128, 10), f32)
    for s in range(8):
        r0 = 64 * (s // 4) + 8 * (s % 4)
        bp[r0:r0 + 8, 0] = -b1
    for k in range(3):
        for s in range(8):
            bp[16 * s:16 * s + 16, 1 + k] = -ba_list[k]
        for a in range(4):
            bp[32 * a:32 * a + 32, 4 + k] = bb_list[k]
            bp[32 * a:32 * a + 32, 7 + k] = -bb_list[k]
    return dict(wa8=wa8, wh16=wh16, wf16=wf16, bp=bp)


def _split_multi_waits(nc):
    """Walrus codegen allows one sync-wait per instruction; hoist extra
    waits onto same-engine NoOps inserted just before the instruction."""
    n = 0
    for fn in nc.m.functions:
        for blk in fn.blocks:
            out = []
            for ins in blk.instructions:
                si = ins.sync_info
                if si is not None and len(si.on_wait) > 1:
                    waits = list(si.on_wait)
                    for j, w in enumerate(waits[:-1]):
                        nop = mybir.InstNoOp(name=f"{ins.name}-wsplit{j}")
                        nop.engine = ins.engine
                        nop.sync_info = mybir.SyncInfo(on_wait=[w],
                                                       on_update=[])
                        nc.register_instruction(nop)
                        out.append(nop)
                        n += 1
                    si.on_wait = [waits[-1]]
                out.append(ins)
            blk.instructions[:] = out
    return n


def build_nc(n_g=N_G_MIN):
    nc = bass.Bass("TRN2", target_bir_lowering=False, debug=False)

    R = 3 * n_g                 # padded rows per core
    MC = R // 4                 # x2 / yT columns

    x2_d = nc.dram_tensor("x2", [128, 2, MC], F8, kind="ExternalInput").ap()
    wa_d = nc.dram_tensor("wa8", [128, 2, 64], F8, kind="ExternalInput").ap()
    wh_d = nc.dram_tensor("wh16", [128, 384], F16, kind="ExternalInput").ap()
    wf_d = nc.dram_tensor("wf16", [128, 384], F16, kind="ExternalInput").ap()
    bp_d = nc.dram_tensor("bp", [128, 10], F32, kind="ExternalInput").ap()
    yt_d = nc.dram_tensor("yT", [128, MC], F16, kind="ExternalOutput").ap()

    TANH = mybir.ActivationFunctionType.Tanh
    SIG = mybir.ActivationFunctionType.Sigmoid
    SUB = mybir.AluOpType.subtract
    IDENT = mybir.ActivationFunctionType.Identity
    DR = mybir.MatmulPerfMode.DoubleRow

    info = _binfo(n_g)
    tiles, owner = _tinfo(n_g)
    nb = len(info)
    nt = len(tiles)
    # blocks of each DMA tile / superstep, and their in-tile column offsets
    tblocks = [[] for _ in range(nt)]
    for b, (t, off, last) in enumerate(owner):
        tblocks[t].append((b, off))
    # trunk PSUM column offset per block: second block of a pair goes at
    # 512 so each DoubleRow dst stays inside one PSUM bank
    soff = {b: (0 if off == 0 else 512) for b, (t, off, last) in
            enumerate(owner)}
    PREFETCH = 3                # supersteps of x-tile lookahead

    def xtile_for(xts, t):
        if t not in xts:
            m0, mlen = tiles[t]
            xts[t] = xpool.tile([128, 2, 2000], F8, name="xt")
            nc.sync.dma_start(xts[t][:, :, :mlen], x2_d[:, :, m0:m0 + mlen])
        return xts[t]

    with tile.TileContext(nc) as tc:
        with (
            tc.tile_pool(name="const", bufs=1) as cpool,
            tc.tile_pool(name="xin", bufs=5) as xpool,
            tc.tile_pool(name="ty", bufs=2) as typ,
            tc.tile_pool(name="tg", bufs=3) as tgp,
            tc.tile_pool(name="outp", bufs=3) as opool,
            tc.tile_pool(name="sta", bufs=1, space="PSUM") as stap,
            tc.tile_pool(name="stb", bufs=1, space="PSUM") as stbp,
            tc.tile_pool(name="sh", bufs=2, space="PSUM") as shp,
            tc.tile_pool(name="pf", bufs=2, space="PSUM") as pfp,
        ):
            xts = {}
            xtile_for(xts, 0)   # first x tile ahead of the const DMAs
            wa = cpool.tile([128, 2, 64], F8)
            nc.sync.dma_start(wa[:], wa_d)
            wh = cpool.tile([128, 384], F16)
            nc.sync.dma_start(wh[:], wh_d)
            wf = cpool.tile([128, 384], F16)
            nc.sync.dma_start(wf[:], wf_d)
            bp = cpool.tile([128, 10], F32)
            nc.sync.dma_start(bp[:], bp_d)
            for t in range(1, min(PREFETCH, nt)):
                xtile_for(xts, t)

            # HAM/pstate warmup: dependency-free matmuls that run while
            # the first x tiles stream in, so the PE clock gate is open
            # when the main loop starts.
            for _ in range(8):
                wps = shp.tile([128, 500], F32, name="warm", tag="sh")
                nc.tensor.matmul(wps[0:16, 0:384], wh[:, 0:16], wh[:],
                                 start=True, stop=True)

            ty2 = {}
            tg = {}
            # Superstep-pipelined emission (superstep = one DMA tile = up
            # to 2 blocks): per step t the PE does trunk(t), mmF(t-2),
            # mmH(t-1) so every PE wait is pre-satisfied.
            for t in range(nt + 2):
                if t < nt:
                    if t + PREFETCH < nt:
                        xtile_for(xts, t + PREFETCH)
                    xt = xts[t]
                    # trunk: 2 DoubleRow matmuls per block into shared
                    # [64, 1024] tiles, then ONE ACT per half-superstep
                    sa = stap.tile([64, 1024], F32, name="sa", tag="sta")
                    sb = stbp.tile([64, 1024], F32, name="sb", tag="stb")
                    wid = 0
                    for b, xo in tblocks[t]:
                        cs = info[b][1]
                        so = soff[b]
                        nc.tensor.matmul(sa[:, so:so + cs], wa[:],
                                         xt[:, :, xo:xo + cs],
                                         start=True, stop=True, perf_mode=DR)
                        nc.tensor.matmul(sb[:, so:so + cs], wa[:],
                                         xt[:, :, xo + cs:xo + 2 * cs],
                                         start=True, stop=True, perf_mode=DR)
                        wid = so + cs
                    ty2[t] = typ.tile([128, 1024], F16, name="ty2")
                    nc.scalar.activation(ty2[t][0:64, :wid], sa[:, :wid],
                                         TANH, bias=bp[0:64, 0:1],
                                         scale=1.0 / (XS * WS))
                    nc.scalar.activation(ty2[t][64:128, :wid], sb[:, :wid],
                                         TANH, bias=bp[64:128, 0:1],
                                         scale=1.0 / (XS * WS))
                    del sa, sb, xts[t]
                if 0 <= t - 2:
                    t2 = t - 2
                    ot = opool.tile([128, 2000], F16, name="ot")
                    for b, oo in tblocks[t2]:
                        k, cs, m0, y0 = info[b]
                        pa = pfp.tile([128, 500], F32, name="pa", tag="pf")
                        nc.tensor.matmul(pa[:, :cs],
                                         wf[0:64, 128 * k:128 * (k + 1)],
                                         tg[b][0:64, :cs],
                                         start=True, stop=True)
                        if t2 >= nt - 2:
                            # drain tail: ACT is idle here -- run half the
                            # bias-subtracts on it so DVE isn't the pacer
                            nc.scalar.activation(ot[:, oo:oo + cs],
                                                 pa[:, :cs], IDENT,
                                                 bias=bp[:, 7 + k:8 + k],
                                                 scale=1.0)
                        else:
                            nc.vector.tensor_scalar(ot[:, oo:oo + cs],
                                                    pa[:, :cs],
                                                    bp[:, 4 + k:5 + k],
                                                    None, SUB)
                        pb = pfp.tile([128, 500], F32, name="pb", tag="pf")
                        nc.tensor.matmul(pb[:, :cs],
                                         wf[64:128, 128 * k:128 * (k + 1)],
                                         tg[b][64:128, :cs],
                                         start=True, stop=True)
                        nc.vector.tensor_scalar(ot[:, oo + cs:oo + 2 * cs],
                                                pb[:, :cs],
                                                bp[:, 4 + k:5 + k], None, SUB)
                        del tg[b], pa, pb
                    tm0, tmlen = tiles[t2]
                    nc.gpsimd.dma_start(yt_d[:, tm0:tm0 + tmlen],
                                        ot[:, :tmlen])
                    del ot
                if 0 <= t - 1 < nt:
                    t1 = t - 1
                    for b, _ in tblocks[t1]:
                        k, cs, m0, y0 = info[b]
                        so = soff[b]
                        sh = shp.tile([128, 500], F32, name="sh", tag="sh")
                        nc.tensor.matmul(sh[:, :cs],
                                         wh[:, 128 * k:128 * (k + 1)],
                                         ty2[t1][:, so:so + cs],
                                         start=True, stop=True)
                        tg[b] = tgp.tile([128, 500], F16, name="tg")
                        nc.scalar.activation(tg[b][:, :cs], sh[:, :cs], SIG,
                                             bias=bp[:, 1 + k:2 + k],
                                             scale=1.0)
                        del sh
                    del ty2[t1]

    _split_multi_waits(nc)
    return nc


_NC_CACHE = {}


def _get_nc(n_g=N_G_MIN):
    if n_g not in _NC_CACHE:
        _NC_CACHE[n_g] = build_nc(n_g)
    return _NC_CACHE[n_g]


_DECODE_CACHE = {}


def _decode_maps(n_g):
    """Per padded-sorted row r: (yT column, yT partition base) arrays."""
    if n_g in _DECODE_CACHE:
        return _DECODE_CACHE[n_g]
    cols = []
    parts = []
    for k, cs, m0, y0 in _binfo(n_g):
        r = np.arange(8 * cs)
        half = r // (4 * cs)        # which trunk matmul / mmF tile
        w = r % (4 * cs)
        t = w >> 2                  # column within chunk
        a = w & 3                   # slot within half
        cols.append(y0 + half * cs + t)
        parts.append(32 * a)
    m = (np.concatenate(cols).astype(np.int32),
         np.concatenate(parts).astype(np.int32))
    _DECODE_CACHE[n_g] = m
    return m


def _choose_n_g(u):
    mx = 0
    for c in range(N_CORES):
        uc = u[c * B_C:(c + 1) * B_C]
        mx = max(mx, int(np.bincount(uc, minlength=3).max()))
    return max(N_G_MIN, 2000 * math.ceil(mx / 2000))


def kernel(x, u, w1, b1, w2, b2, w3, b3, w4, b4, w5, b5, w6, b6, w7, b7):
    global _LAST_RES
    x = np.ascontiguousarray(np.asarray(x, np.float32))
    u = np.ascontiguousarray(np.asarray(u, np.int32))
    weights = [np.asarray(t, np.float32) for t in
               (w1, b1, w2, b2, w3, b3, w4, b4, w5, b5, w6, b6, w7, b7)]

    n_g = _choose_n_g(u)
    R = 3 * n_g
    nc = _get_nc(n_g)
    packed = _pack_weights(*weights)

    in_maps = []
    idx_all = []
    for c in range(N_CORES):
        xc = x[c * B_C:(c + 1) * B_C]
        uc = u[c * B_C:(c + 1) * B_C]
        idx_k = [np.flatnonzero(uc == k) for k in range(3)]
        idx_all.append(idx_k)
        xs = np.zeros((R, IN), np.float32)
        for k in range(3):
            xs[k * n_g:k * n_g + len(idx_k[k])] = xc[idx_k[k]]
        xs *= XS
        xq = xs.astype(E4)
        # x2[64a+f, i, m] = XS * x[4m + 2i + a, f]
        x2 = xq.reshape(R // 4, 2, 2, IN).transpose(2, 3, 1, 0)
        x2 = x2.reshape(128, 2, R // 4)
        in_maps.append({"x2": np.ascontiguousarray(x2), **packed})

    res = run_bass_kernel_spmd(nc, in_maps, core_ids=list(range(N_CORES)),
                               trace=_TRACE)
    _LAST_RES = res

    cols, parts = _decode_maps(n_g)
    gather_p = parts[:, None] + np.arange(OUT, dtype=np.int32)[None, :]
    y = np.zeros((B, OUT), np.float32)
    for c in range(N_CORES):
        yt = res.results[c]["yT"]
        ys = yt[gather_p, cols[:, None]].astype(np.float32)
        yc = y[c * B_C:(c + 1) * B_C]
        for k in range(3):
            yc[idx_all[c][k]] = ys[k * n_g:k * n_g + len(idx_all[c][k])]
    return y
